# revision 48
# baseline (speedup 1.0000x reference)
"""CIN (Compressed Interaction Network) forward kernel for 8 Trainium2 NeuronCores.

Reference computation (per batch b, embedding dim d):
    x0 = inputs[b, :, d]                 # [F=39]
    h0 = x0
    for k in 0..2:
        z  = outer(x0, h_{k})            # [F * Hk]
        h_{k+1} = z @ Wk + bk            # [256]
    out[b] = concat_k sum_d h_{k+1}      # [768]

Strategy: data-parallel over batch (64 per core); per core the GEMM rows
r = (b, d) are 2048 columns, laid out transposed (x0T[f, r], hT[u, r]).

The whole network is independent per batch, so the kernel runs as a
4-stage SOFTWARE PIPELINE over batch groups of 16 (512 columns each):

  L0(g0) | L1(g0)+L0(g1) | L1(g1)+L0(g2)+T/G(g0) | L1(g2)+L0(g3)+T/G(g1) |
         |  L1(g3)+T/G(g2) | tail: T/G(g3) + W2 contraction

where L0 is the layer-0 GEMM over 780 symmetric x-pair rows (z0 products
host-built, streamed as fp16 quarter-tiles; W0 rows folded W0[i,j]+W0[j,i]),
L1 is the full layer-1 GEMM (z built on the Vector engine from DMA-broadcast
x0 rows), and T/G is layer 2's u->d StreamTranspose + per-batch Gram
matmuls G2[b,i,j] = sum_d x0[b,i,d] h2[b,j,d] (layer 2's feature map is only
used summed over d, so the 10.5-GFLOP layer-2 GEMM collapses to Grams plus a
small W2 contraction out2[u,b] = sum_{i,j} W2[(i,j),u] G2[b,i,j]).

Everything except the first L0 group and the last T/G group hides inside the
L1 matmul stream, which also keeps the HAM clock governor at full rate.
Each group owns one PSUM bank pair (ps_{c}_{g}); gram waves and the W2
contraction reuse banks of retired groups.
"""

import os
import sys

import numpy as np

for _p in ("/opt/trn_rl_repo", "/root/.axon_site/_ro/trn_rl_repo"):
    if os.path.isdir(_p) and _p not in sys.path:
        sys.path.insert(0, _p)

N_CORES = 8
B, F, D = 512, 39, 32
U = 256
BL = B // N_CORES          # 64 batches per core
R = BL * D                 # 2048 GEMM rows per core
NG = 4                     # pipeline groups
RG = R // NG               # 512 columns (16 batches) per group
BG = BL // NG              # 16 batches per group
NP = F * (F + 1) // 2      # 780 symmetric pairs for layer 0
KT0 = 7                    # layer-0 k-tiles: 6x128 + 1x12
KL0 = [128] * 6 + [NP - 768]
K12 = F * U                # 9984
KT12 = K12 // 128          # 78 k-tiles; kt = (i, half)
NWB = 8                    # gram matmuls (batches) per PSUM wave

DT = "float16"             # device compute dtype for z / W / h

_prog_cache = {}


def _np_dt():
    import ml_dtypes

    return np.float16 if DT == "float16" else ml_dtypes.bfloat16


def _build_program():
    import concourse.mybir as mybir
    from concourse import bacc, tile

    dt = mybir.dt
    cdt = getattr(dt, DT)
    f32 = dt.float32

    nc = bacc.Bacc(
        "TRN2", target_bir_lowering=False, debug=False, num_devices=N_CORES
    )
    z0_p = nc.declare_dram_parameter("z0", [128, KT0, R], cdt, isOutput=False)
    x0r_p = nc.declare_dram_parameter("x0r", [F * 32, R], cdt, isOutput=False)
    x0d_p = nc.declare_dram_parameter("x0d", [32, BL, F], cdt, isOutput=False)
    w0_p = nc.declare_dram_parameter("w0", [128, KT0, U], cdt, isOutput=False)
    w1_p = nc.declare_dram_parameter("w1", [128, KT12, U], cdt, isOutput=False)
    w2_p = nc.declare_dram_parameter("w2", [128, KT12, U], cdt, isOutput=False)
    bias_p = nc.declare_dram_parameter("bias", [128, 4], f32, isOutput=False)
    out_p = nc.declare_dram_parameter("out", [128, 6, BL], f32, isOutput=True)

    with tile.TileContext(nc) as tc:
        with (
            tc.tile_pool(name="const", bufs=1) as constp,
            tc.tile_pool(name="wpool", bufs=1) as wpool,
            tc.tile_pool(name="xb", bufs=6) as xbp,
            tc.tile_pool(name="z0p", bufs=7) as z0p,
            tc.tile_pool(name="zp", bufs=3) as zp,
            tc.tile_pool(name="hp", bufs=1) as hp,
            tc.tile_pool(name="psum", bufs=1, space="PSUM") as psp,
        ):
            bcast_n = [0]

            def bcast(dst, src_ap):
                eng = nc.sync if bcast_n[0] % 2 == 0 else nc.scalar
                bcast_n[0] += 1
                eng.dma_start(dst, src_ap)

            out_sb = constp.tile([128, 6, BL], f32, tag="out")
            h_tiles = {
                (l, c): hp.tile([128, R], cdt, tag=f"h{l}{c}", name=f"h{l}{c}")
                for l in range(2)
                for c in range(2)
            }
            h2d = hp.tile([32, 2, BL, 128], cdt, tag="h2d", name="h2d")
            g2 = hp.tile([128, 2, F, BL], cdt, tag="g2", name="g2")
            x0d = constp.tile([32, BL, F], cdt, tag="x0d")

            w0 = wpool.tile([128, KT0, U], cdt, tag="w0")
            w1 = wpool.tile([128, KT12, U], cdt, tag="w1")
            w2 = wpool.tile([128, KT12, U], cdt, tag="w2")
            bias = constp.tile([128, 4], f32, tag="bias")

            # ---- prologue DMA: group-0's z0 quarters + W0 + the first two
            # W1 k-tiles are the startup critical path; everything else
            # follows them in ring-FIFO order.
            z0_tiles = [
                z0p.tile([128, R], cdt, tag="z0", name=f"z0_{t}") for t in range(KT0)
            ]

            def z0_dma(g, eng):
                for t in range(KT0):
                    eng.dma_start(
                        z0_tiles[t][: KL0[t], g * RG : (g + 1) * RG],
                        z0_p[: KL0[t], t, g * RG : (g + 1) * RG],
                    )

            nc.scalar.dma_start(w0[:, :, :], w0_p[:, :, :])
            nc.scalar.dma_start(w1[:, 0:2, :], w1_p[:, 0:2, :])
            z0_dma(0, nc.sync)
            nc.scalar.dma_start(bias[:, :], bias_p[:, :])
            w1_chunks = list(range(0, KT12, 13))

            # PE warm-up: covers group-0 z0 landing, spins up the HAM clock
            warm_ps = psp.tile([128, RG], f32, tag="ps_0_0", name="warm_ps")
            nc.vector.memset(h_tiles[(0, 0)][:, :RG], 0)
            for _ in range(10):
                nc.tensor.matmul(
                    warm_ps[:, :],
                    h_tiles[(0, 0)][:, :128],
                    h_tiles[(0, 0)][:, :RG],
                    start=True,
                    stop=True,
                )

            def make_x(g, i, nm, eng=None):
                t = xbp.tile([128, RG], cdt, tag="xi", name=nm, bufs=6)
                src = (
                    x0r_p[i * 32 : i * 32 + 32, g * RG : (g + 1) * RG]
                    .unsqueeze(1)
                    .to_broadcast((32, 4, RG))
                )
                if eng is None:
                    bcast(t[:, :], src)
                else:
                    eng.dma_start(t[:, :], src)
                return t

            x_pre = {(0, 0): make_x(0, 0, "xg0i0", eng=nc.sync)}
            x_pre[(0, 1)] = make_x(0, 1, "xg0i1", eng=nc.scalar)
            nc.scalar.dma_start(w1[:, 2:13, :], w1_p[:, 2:13, :])
            z0_dma(1, nc.sync)
            nc.sync.dma_start(w1[:, 13:26, :], w1_p[:, 13:26, :])

            # ---- building blocks ----
            def l0_group(g, start_kt, end_kt):
                # layer-0 matmuls for column group g into ps_{c}_{g}
                ps = [
                    psp.tile([128, RG], f32, tag=f"ps_{c}_{g}", name=f"l0ps{g}{c}")
                    for c in range(2)
                ]
                for kt in range(start_kt, end_kt):
                    klen = KL0[kt]
                    for c in range(2):
                        nc.tensor.matmul(
                            ps[c][:, :],
                            w0[:klen, kt, c * 128 : (c + 1) * 128],
                            z0_tiles[kt][:klen, g * RG : (g + 1) * RG],
                            start=(kt == 0),
                            stop=(kt == KT0 - 1),
                        )
                return ps

            def l0_evac(g, ps):
                for c in range(2):
                    dst = h_tiles[(0, c)][:, g * RG : (g + 1) * RG]
                    if c == 0:
                        nc.vector.tensor_scalar_add(dst, ps[0][:, :], bias[:, 0:1])
                    else:
                        nc.scalar.activation(
                            dst, ps[1][:, :],
                            mybir.ActivationFunctionType.Identity,
                            bias=bias[:, 1:2],
                        )

            def emit_st(g, h, a):
                nc.vector.transpose(
                    h2d[:, h, BG * g : BG * (g + 1), 32 * a : 32 * (a + 1)],
                    h_tiles[(1, h)][32 * a : 32 * (a + 1), g * RG : (g + 1) * RG],
                )

            def gram_wave(h, bg, filler, tag):
                pt = psp.tile(
                    [128, NWB * F], f32, tag=tag, name=f"gps{h}_{bg}",
                )
                for _ in range(filler):
                    nc.tensor.matmul(
                        pt[:, : NWB * F],
                        h_tiles[(0, 0)][:, :128],
                        h_tiles[(0, 0)][:, : NWB * F],
                        start=True,
                        stop=True,
                    )
                for g in range(NWB):
                    b = bg * NWB + g
                    nc.tensor.matmul(
                        pt[:, g * F : (g + 1) * F],
                        h2d[:, h, b, :],
                        x0d[:, b, :],
                        start=True,
                        stop=True,
                    )
                nc.scalar.activation(
                    g2[:, h, :, bg * NWB : (bg + 1) * NWB].rearrange(
                        "p i b -> p b i"
                    ),
                    pt[:, :].rearrange("p (b i) -> p b i", i=F),
                    mybir.ActivationFunctionType.Identity,
                )

            def h_reduce(l):
                for c in range(2):
                    nc.vector.tensor_reduce(
                        out_sb[:, l * 2 + c, :],
                        h_tiles[(l, c)].rearrange("p (b d) -> p b d", d=D),
                        axis=mybir.AxisListType.X,
                        op=mybir.AluOpType.add,
                    )

            # ---- layer 1 for one group, with a per-kt hook ----
            def l1_group(g, z_pre, kt_hook, pre_evac=None):
                ps = [
                    psp.tile([128, RG], f32, tag=f"ps_{c}_{g}", name=f"l1ps{g}{c}")
                    for c in range(2)
                ]
                xcur = [None]
                for kt in range(KT12):
                    if kt_hook is not None:
                        kt_hook(kt)
                    i, half = kt // 2, kt % 2
                    if half == 0:
                        xcur[0] = (
                            x_pre.pop((g, i)) if (g, i) in x_pre
                            else make_x(g, i, f"x{g}_{i}")
                        )
                    if kt in z_pre:
                        z_t = z_pre.pop(kt)
                    else:
                        z_t = zp.tile([128, RG], cdt, tag="z", name="zs")
                        nc.vector.tensor_mul(
                            z_t[:, :],
                            xcur[0][:, :],
                            h_tiles[(0, half)][:, g * RG : (g + 1) * RG],
                        )
                    for c in range(2):
                        nc.tensor.matmul(
                            ps[c][:, :],
                            w1[:, kt, c * 128 : (c + 1) * 128],
                            z_t[:, :],
                            start=(kt == 0),
                            stop=(kt == KT12 - 1),
                        )
                if pre_evac is not None:
                    pre_evac()
                for c in range(2):
                    dst = h_tiles[(1, c)][:, g * RG : (g + 1) * RG]
                    if c == 0:
                        nc.vector.tensor_scalar_add(dst, ps[0][:, :], bias[:, 2:3])
                    else:
                        nc.scalar.activation(
                            dst, ps[1][:, :],
                            mybir.ActivationFunctionType.Identity,
                            bias=bias[:, 3:4],
                        )

            # weight-chunk streaming schedule: (phase, kt) -> (tile, src, chunk)
            # w1's remaining chunks load EARLY in phase 0 (it consumes them at
            # one k-tile per ~0.4us); w2 is needed only by the very tail.
            w_sched = {
                (0, 2): (w1, w1_p, 2), (0, 6): (w1, w1_p, 3),
                (0, 10): (w1, w1_p, 4), (0, 14): (w1, w1_p, 5),
                (1, 8): (w2, w2_p, 0), (1, 40): (w2, w2_p, 1),
                (2, 8): (w2, w2_p, 2), (2, 40): (w2, w2_p, 3),
                (3, 2): (w2, w2_p, 4), (3, 20): (w2, w2_p, 5),
            }
            # wave PSUM tags per phase: only banks of retired groups (the
            # running L1 group g and the hooked L0 group g+1 own theirs)
            PHASE_WTAGS = {
                1: ["ps_0_0", "ps_1_0", "ps_0_3", "ps_1_3"],
                2: ["ps_0_1", "ps_1_1", "ps_0_0", "ps_1_0"],
                3: ["ps_0_2", "ps_1_2", "ps_0_1", "ps_1_1"],
            }
            # layer-0 matmuls for group g+1 hook into phase g
            L0_KTS = {30: (0, 2), 34: (2, 4), 38: (4, 6), 42: (6, 7)}
            ST_KTS = {4: (0, 0), 10: (0, 1), 16: (0, 2), 22: (0, 3),
                      28: (1, 0), 34: (1, 1), 40: (1, 2), 46: (1, 3)}
            WV_KTS = {52: 0, 58: 1, 64: 2, 70: 3}   # (h, wave) = divmod

            z_pre_next = [{}]

            def phase_hook(g):
                l0_ps = [None]
                l0_parts = {}

                def hook(kt):
                    if (g, kt) in w_sched:
                        wt, wp, c = w_sched[(g, kt)]
                        lo = w1_chunks[c]
                        (nc.sync if c % 2 else nc.scalar).dma_start(
                            wt[:, lo : lo + 13, :], wp[:, lo : lo + 13, :]
                        )
                    if g < NG - 1 and kt in L0_KTS:
                        a, bnd = L0_KTS[kt]
                        if a == 0:
                            l0_parts["ps"] = l0_group(g + 1, 0, bnd)
                        else:
                            gg = g + 1
                            for t in range(a, bnd):
                                klen = KL0[t]
                                for c in range(2):
                                    nc.tensor.matmul(
                                        l0_parts["ps"][c][:, :],
                                        w0[:klen, t, c * 128 : (c + 1) * 128],
                                        z0_tiles[t][:klen, gg * RG : (gg + 1) * RG],
                                        start=False,
                                        stop=(t == KT0 - 1),
                                    )
                        if bnd == KT0:
                            l0_evac(g + 1, l0_parts["ps"])
                    if g >= 1 and kt in ST_KTS:
                        h, a = ST_KTS[kt]
                        emit_st(g - 1, h, a)
                    if g >= 1 and kt in WV_KTS:
                        idx = WV_KTS[kt]
                        h, w = divmod(idx, 2)
                        gram_wave(h, 2 * (g - 1) + w, 0, PHASE_WTAGS[g][idx])
                    if g == 0 and kt == 20:
                        z0_dma(2, nc.scalar)
                    if g == 0 and kt == 44:
                        z0_dma(3, nc.sync)
                    if g == 0 and kt == 50:
                        nc.scalar.dma_start(x0d[:, :, :], x0d_p[:, :, :])
                    if g == NG - 1 and kt == 48:
                        h_reduce(0)
                    if g == NG - 1 and kt == 52:
                        nc.gpsimd.dma_start(out_p[:, 0:2, :], out_sb[:, 0:2, :])
                    if kt == 70 and g < NG - 1:
                        x_pre[(g + 1, 0)] = make_x(g + 1, 0, f"xg{g + 1}i0")
                    if kt == 74 and g < NG - 1:
                        x_pre[(g + 1, 1)] = make_x(g + 1, 1, f"xg{g + 1}i1")

                return hook

            def pre_evac(g):
                def fn():
                    if g >= NG - 1:
                        return
                    for kt in range(2):   # both kt 0, 1 use x0 row i=0
                        z_t = zp.tile([128, RG], cdt, tag="z", name=f"zn{g}{kt}")
                        nc.vector.tensor_mul(
                            z_t[:, :],
                            x_pre[(g + 1, 0)][:, :],
                            h_tiles[(0, kt % 2)][
                                :, (g + 1) * RG : (g + 2) * RG
                            ],
                        )
                        z_pre_next[0][kt] = z_t

                return fn

            # ---- the pipeline ----
            ps_g0 = l0_group(0, 0, KT0)
            l0_evac(0, ps_g0)
            for g in range(NG):
                zp_in = z_pre_next[0]
                z_pre_next[0] = {}
                l1_group(g, zp_in, phase_hook(g), pre_evac(g))

            # ---- exposed tail: group-3 transposes + grams + W2 contraction ----
            for h in range(2):
                for a in range(4):
                    emit_st(NG - 1, h, a)

            ps_f = [
                psp.tile([128, BL], f32, tag="ps_0_3", name="psf0"),
                psp.tile([128, BL], f32, tag="ps_1_3", name="psf1"),
            ]

            def final_half(hi, h):
                for i in range(F):
                    for uh in range(2):
                        nc.tensor.matmul(
                            ps_f[uh][:, :],
                            w2[:, h * F + i, uh * 128 : (uh + 1) * 128],
                            g2[:, h, i, :],
                            start=(hi == 0 and i == 0),
                            stop=(hi == 1 and i == F - 1),
                        )

            gram_wave(0, 2 * (NG - 1), 2, "ps_0_2")
            gram_wave(0, 2 * (NG - 1) + 1, 1, "ps_1_2")
            final_half(0, 0)
            gram_wave(1, 2 * (NG - 1), 1, "ps_0_0")
            gram_wave(1, 2 * (NG - 1) + 1, 0, "ps_1_0")
            final_half(1, 1)

            h_reduce(1)
            nc.gpsimd.dma_start(out_p[:, 2:4, :], out_sb[:, 2:4, :])
            for uh in range(2):
                nc.vector.tensor_copy(out_sb[:, 4 + uh, :], ps_f[uh][:, :])
            nc.sync.dma_start(out_p[:, 4:6, :], out_sb[:, 4:6, :])

    nc.compile()
    return nc


def _get_program():
    if "nc" not in _prog_cache:
        _prog_cache["nc"] = _build_program()
    return _prog_cache["nc"]


def _prep_maps(inputs):
    cdt = _np_dt()
    x = np.asarray(inputs["inputs"], np.float32)          # [512, 39, 32]
    Ws = [np.asarray(inputs[f"W{k}"], np.float32) for k in range(3)]
    bs = [np.asarray(inputs[f"b{k}"], np.float32) for k in range(3)]

    ii, jj = np.triu_indices(F)                           # 780 pairs, i-major

    w0r = Ws[0].reshape(F, F, U)
    w0s = np.where((ii == jj)[:, None], w0r[ii, jj], w0r[ii, jj] + w0r[jj, ii])
    w0t = np.zeros((KT0 * 128, U), np.float32)
    w0t[:NP] = w0s
    w_tiled = [
        w0t.reshape(KT0, 128, U).transpose(1, 0, 2).astype(cdt),
        Ws[1].reshape(KT12, 128, U).transpose(1, 0, 2).astype(cdt),
        # W2 relayout for the gram contraction: [(i, j), u] ->
        # [j%128, (j//128)*F + i, u]
        Ws[2].reshape(F, 2, 128, U).transpose(2, 1, 0, 3).reshape(128, KT12, U)
        .astype(cdt),
    ]
    w_tiled = [np.ascontiguousarray(w) for w in w_tiled]
    bias = np.zeros((128, 4), np.float32)
    for l in range(2):
        for c in range(2):
            bias[:, l * 2 + c] = bs[l][c * 128 : (c + 1) * 128]

    in_maps = []
    for core in range(N_CORES):
        xs = x[core * BL : (core + 1) * BL]               # [64, 39, 32]
        x0T = xs.transpose(1, 0, 2).reshape(F, R)         # fp32 [39, 2048]
        z0 = np.zeros((KT0 * 128, R), np.float32)
        z0[:NP] = x0T[ii] * x0T[jj]
        z0t = np.ascontiguousarray(
            z0.reshape(KT0, 128, R).transpose(1, 0, 2).astype(cdt)
        )
        x0r = np.ascontiguousarray(np.repeat(x0T.astype(cdt), 32, axis=0))
        x0d = np.ascontiguousarray(xs.transpose(2, 0, 1).astype(cdt))
        in_maps.append(
            {
                "z0": z0t,
                "x0r": x0r,
                "x0d": x0d,
                "w0": w_tiled[0],
                "w1": w_tiled[1],
                "w2": w_tiled[2],
                "bias": bias,
            }
        )
    return in_maps, bs


def _finish_output(results, bs):
    outs = []
    for core in range(N_CORES):
        o = np.asarray(results[core]["out"], np.float32)  # [128, 6, 64]
        outs.append(o.transpose(2, 1, 0).reshape(BL, 768))
    out = np.concatenate(outs, axis=0)
    for l in range(3):
        out[:, l * U : (l + 1) * U] += D * bs[l]
    return np.ascontiguousarray(out.astype(np.float32))


def kernel(**inputs) -> np.ndarray:
    from concourse.bass_utils import run_bass_kernel_spmd

    in_maps, bs = _prep_maps(inputs)
    nc = _get_program()
    res = run_bass_kernel_spmd(nc, in_maps, list(range(N_CORES))).results
    return _finish_output(res, bs)


# revision 51
# speedup vs baseline: 1.0864x; 1.0864x over previous
"""CIN (Compressed Interaction Network) forward kernel for 8 Trainium2 NeuronCores.

Reference computation (per batch b, embedding dim d):
    x0 = inputs[b, :, d]                 # [F=39]
    h0 = x0
    for k in 0..2:
        z  = outer(x0, h_{k})            # [F * Hk]
        h_{k+1} = z @ Wk + bk            # [256]
    out[b] = concat_k sum_d h_{k+1}      # [768]

Strategy: data-parallel over batch (64 per core).  Per core, rows r = (b, d)
are 2048 GEMM rows.  Everything is laid out transposed: x0T[f, r], hT[u, r].

Layer 0 exploits z0 symmetry (x_i x_j = x_j x_i): only the 780 upper-triangle
pairs are kept, with W0 rows folded (W0[i,j] + W0[j,i] off-diagonal), so K
drops 1521 -> 780 (7 k-tiles instead of 13).  The pair products are built on
the host in fp32 and streamed to the device as fp16 tiles; they are the
startup critical path and are split across both HWDGE rings.

Layer 1 is the full GEMM: z1[(i,j), r] = x0[i, r] * h1[j, r] built k-tile by
k-tile on the Vector engine from DMA-broadcast x0 rows.  It runs in TWO
column groups (batches 0-31, 32-63): group 0 finishes its whole K loop
first, so its h2 evacuation, u->d stream transposes, and gram matmuls are
hooked into group 1's matmul stream, where the big matmuls keep the HAM
clock governor at full rate and the small ops hide completely.

Layer 2's feature map is only ever used summed over d, so the full GEMM is
replaced by per-batch Grams: G2[b,i,j] = sum_d x0[b,i,d] h2[b,j,d], then
out2[u,b] = sum_{i,j} W2[(i,j),u] G2[b,i,j].  Only the second batch group's
transposes/grams plus the final 156-matmul W2 contraction remain exposed
after layer 1, with filler matmuls holding the clock at full rate.
"""

import os
import sys

import numpy as np

for _p in ("/opt/trn_rl_repo", "/root/.axon_site/_ro/trn_rl_repo"):
    if os.path.isdir(_p) and _p not in sys.path:
        sys.path.insert(0, _p)

N_CORES = 8
B, F, D = 512, 39, 32
U = 256
BL = B // N_CORES          # 64 batches per core
R = BL * D                 # 2048 GEMM rows per core
RG = R // 2                # layer-1 column group width (32 batches)
NB = 512                   # matmul moving free-dim (one PSUM bank of fp32)
NRB = R // NB              # 4 row blocks
NP = F * (F + 1) // 2      # 780 symmetric pairs for layer 0
KT0 = 7                    # layer-0 k-tiles: 6x128 + 1x12
KL0 = [128] * 6 + [NP - 768]
K12 = F * U                # 9984
KT12 = K12 // 128          # 78 k-tiles; kt = (i, half)
NWB = 8                    # gram matmuls (batches) per PSUM wave

DT = "float16"             # device compute dtype for z / W / h ("float16" | "bfloat16")

_prog_cache = {}


def _np_dt():
    import ml_dtypes

    return np.float16 if DT == "float16" else ml_dtypes.bfloat16


def _build_program():
    import concourse.mybir as mybir
    from concourse import bacc, tile

    dt = mybir.dt
    cdt = getattr(dt, DT)
    f32 = dt.float32

    nc = bacc.Bacc(
        "TRN2", target_bir_lowering=False, debug=False, num_devices=N_CORES
    )
    z0_p = nc.declare_dram_parameter("z0", [128, KT0, R], cdt, isOutput=False)
    x0d2_p = nc.declare_dram_parameter("x0d2", [64, 32, F], cdt, isOutput=False)
    ident_p = nc.declare_dram_parameter("ident", [128, 128], cdt, isOutput=False)
    # x0 rows each replicated 32x in DRAM: broadcast DMAs read distinct
    # addresses (HBM bank spread) instead of hammering one 4KB row.
    x0r_p = nc.declare_dram_parameter("x0r", [F * 32, R], cdt, isOutput=False)
    x0d_p = nc.declare_dram_parameter("x0d", [32, BL, F], cdt, isOutput=False)
    w0_p = nc.declare_dram_parameter("w0", [128, KT0, U], cdt, isOutput=False)
    w1_p = nc.declare_dram_parameter("w1", [128, KT12, U], cdt, isOutput=False)
    w2_p = nc.declare_dram_parameter("w2", [128, KT12, U], cdt, isOutput=False)
    bias_p = nc.declare_dram_parameter("bias", [128, 4], f32, isOutput=False)
    out_p = nc.declare_dram_parameter("out", [128, 6, BL], f32, isOutput=True)

    with tile.TileContext(nc) as tc:
        with (
            tc.tile_pool(name="const", bufs=1) as constp,
            tc.tile_pool(name="wpool", bufs=1) as wpool,
            tc.tile_pool(name="xb", bufs=4) as xbp,
            tc.tile_pool(name="z0p", bufs=7) as z0p,
            tc.tile_pool(name="zp", bufs=3) as zp,
            tc.tile_pool(name="hp", bufs=1) as hp,
            tc.tile_pool(name="psum", bufs=1, space="PSUM") as psp,
        ):
            bcast_n = [0]

            def bcast(dst, src_ap):
                eng = nc.sync if bcast_n[0] % 2 == 0 else nc.scalar
                bcast_n[0] += 1
                eng.dma_start(dst, src_ap)

            out_sb = constp.tile([128, 6, BL], f32, tag="out")
            h_tiles = {
                (l, c): hp.tile([128, R], cdt, tag=f"h{l}{c}", name=f"h{l}{c}")
                for l in range(2)
                for c in range(2)
            }
            # layer-2 gram-path tiles: h2d[d, h, b, u_sub]
            h2d = hp.tile([32, 2, BL, 128], cdt, tag="h2d", name="h2d")
            # PE-transposed variant for the tail's h=1 half: partitions are
            # (b_local(2), d) interleaved as the PE transpose emits them
            # (2-batch blocks keep operand base partitions at 0/32; the PE
            # cannot source operands based at partition 96)
            h2dp = hp.tile([64, 16, 128], cdt, tag="h2dp", name="h2dp")
            g2 = hp.tile([128, 2, F, BL], cdt, tag="g2", name="g2")
            x0d = constp.tile([32, BL, F], cdt, tag="x0d")
            x0d2 = constp.tile([64, 32, F], cdt, tag="x0d2")
            ident = constp.tile([128, 128], cdt, tag="ident")

            w0 = wpool.tile([128, KT0, U], cdt, tag="w0")
            w1 = wpool.tile([128, KT12, U], cdt, tag="w1")
            w2 = wpool.tile([128, KT12, U], cdt, tag="w2")
            bias = constp.tile([128, 4], f32, tag="bias")

            # ---- prologue: the layer-0 z tiles are the startup critical
            # path; split them over both HWDGE rings for full aggregate
            # bandwidth.  Ring FIFO keeps later loads from stealing from z0.
            # w1's first two k-tiles go right after w0 so layer 1 can start
            # the moment layer 0 drains (the rest of chunk 0 follows later).
            z0_tiles = [
                z0p.tile([128, R], cdt, tag="z0", name=f"z0_{t}") for t in range(KT0)
            ]
            nc.scalar.dma_start(w0[:, :, :], w0_p[:, :, :])
            nc.scalar.dma_start(w1[:, 0:2, :], w1_p[:, 0:2, :])
            for t in range(KT0 - 1):
                eng = nc.sync if t % 2 == 0 else nc.scalar
                eng.dma_start(z0_tiles[t][:, :], z0_p[:, t, :])
            # last tile holds only 12 live pair rows; don't stream the pad
            nc.sync.dma_start(z0_tiles[6][: KL0[6], :], z0_p[: KL0[6], 6, :])
            nc.scalar.dma_start(bias[:, :], bias_p[:, :])

            # ---- PE warm-up: covers z0[0]+w0 DMA landing and spins the HAM
            # clock gate up (needs ~3.4us sustained matmul activity).
            warm_ps = psp.tile([128, NB], f32, tag="ps_0_0", name="warm_ps")
            nc.vector.memset(h_tiles[(0, 0)][:, :NB], 0)
            for _ in range(10):
                nc.tensor.matmul(
                    warm_ps[:, :],
                    h_tiles[(0, 0)][:, :128],
                    h_tiles[(0, 0)][:, :NB],
                    start=True,
                    stop=True,
                )

            def make_x(g, i, nm, eng=None):
                t = xbp.tile([128, RG], cdt, tag="xi", name=nm, bufs=6)
                src = (
                    x0r_p[i * 32 : i * 32 + 32, g * RG : (g + 1) * RG]
                    .unsqueeze(1)
                    .to_broadcast((32, 4, RG))
                )
                if eng is None:
                    bcast(t[:, :], src)
                else:
                    eng.dma_start(t[:, :], src)
                return t

            # layer-1 group-0 head tiles + the rest of W1 chunk 0 ride the
            # queues BEHIND z0: z0 keeps priority, these land in time.
            l1_pre = {(0, 0): make_x(0, 0, "l1xA0", eng=nc.sync)}
            l1_pre[(0, 1)] = make_x(0, 1, "l1xA1", eng=nc.scalar)
            w1_chunks = list(range(0, KT12, 13))
            nc.scalar.dma_start(w1[:, 2:13, :], w1_p[:, 2:13, :])

            # ---- layer 0: symmetric-pair z streamed from DRAM, full R ----
            ps0 = [
                [
                    psp.tile([128, NB], f32, tag=f"ps_{c}_{r}", name=f"l0ps{c}{r}")
                    for r in range(NRB)
                ]
                for c in range(2)
            ]
            for kt in range(KT0):
                klen = KL0[kt]
                for c in range(2):
                    lhsT = w0[:klen, kt, c * 128 : (c + 1) * 128]
                    for r in range(NRB):
                        nc.tensor.matmul(
                            ps0[c][r][:, :],
                            lhsT,
                            z0_tiles[kt][:klen, r * NB : (r + 1) * NB],
                            start=(kt == 0),
                            stop=(kt == KT0 - 1),
                        )
            for r in range(NRB):
                for c in range(2):
                    if c == 0:
                        nc.vector.tensor_scalar_add(
                            h_tiles[(0, 0)][:, r * NB : (r + 1) * NB],
                            ps0[0][r][:, :],
                            bias[:, 0:1],
                        )
                    else:
                        nc.scalar.activation(
                            h_tiles[(0, 1)][:, r * NB : (r + 1) * NB],
                            ps0[1][r][:, :],
                            mybir.ActivationFunctionType.Identity,
                            bias=bias[:, 1:2],
                        )

            def h_reduce(l):
                for c in range(2):
                    nc.vector.tensor_reduce(
                        out_sb[:, l * 2 + c, :],
                        h_tiles[(l, c)].rearrange("p (b d) -> p b d", d=D),
                        axis=mybir.AxisListType.X,
                        op=mybir.AluOpType.add,
                    )

            # ---- layer-2 building blocks (emitted via hooks) ----
            def emit_st(g, h, a):
                # u->d transpose of one 32-u-row block of batch group g
                nc.vector.transpose(
                    h2d[:, h, 32 * g : 32 * (g + 1), 32 * a : 32 * (a + 1)],
                    h_tiles[(1, h)][32 * a : 32 * (a + 1), g * RG : (g + 1) * RG],
                )

            wave_tags = ["ps_0_0", "ps_0_1", "ps_1_0", "ps_1_1"]
            wv_n = [0]

            def gram_wave(h, bg, filler, pe_path=False):
                pt = psp.tile(
                    [128, NWB * F], f32,
                    tag=wave_tags[wv_n[0] % 4], name=f"gps{h}_{bg}",
                )
                wv_n[0] += 1
                # filler matmuls keep the HAM clock governor at full rate
                # through the exposed small-matmul tail; start=True on the
                # real grams below discards the garbage.
                for _ in range(filler):
                    nc.tensor.matmul(
                        pt[:, : NWB * F],
                        h_tiles[(0, 0)][:, :128],
                        h_tiles[(0, 0)][:, : NWB * F],
                        start=True,
                        stop=True,
                    )
                for g in range(NWB):
                    b = bg * NWB + g
                    if pe_path:
                        # PE-transposed h2 layout: partitions are (b%2, d)
                        bl = b % 2
                        lhsT = h2dp[32 * bl : 32 * (bl + 1), (b - 32) // 2, :]
                        rhs = x0d2[32 * bl : 32 * (bl + 1), b // 2, :]
                    else:
                        lhsT = h2d[:, h, b, :]
                        rhs = x0d[:, b, :]
                    nc.tensor.matmul(
                        pt[:, g * F : (g + 1) * F],
                        lhsT,
                        rhs,
                        start=True,
                        stop=True,
                    )
                # psum wave -> G2 sbuf on the otherwise-idle Scalar engine
                nc.scalar.activation(
                    g2[:, h, :, bg * NWB : (bg + 1) * NWB].rearrange(
                        "p i b -> p b i"
                    ),
                    pt[:, :].rearrange("p (b i) -> p b i", i=F),
                    mybir.ActivationFunctionType.Identity,
                )

            # ---- layer 1, one batch-column group ----
            def layer1_group(g, x_pre, z_pre, kt_hook, pre_evac=None):
                ps = {
                    (c, rr): psp.tile(
                        [128, NB], f32, tag=f"ps_{c}_{2 * g + rr}",
                        name=f"l1ps{g}_{c}{rr}",
                    )
                    for c in range(2)
                    for rr in range(2)
                }
                xcur = [None]
                for kt in range(KT12):
                    if kt_hook is not None:
                        kt_hook(kt)
                    i, half = kt // 2, kt % 2
                    if half == 0:
                        xcur[0] = (
                            x_pre[(g, i)] if (g, i) in x_pre
                            else make_x(g, i, f"x{g}_{i}")
                        )
                    if kt in z_pre:
                        z_t = z_pre[kt]
                    elif g == 0 and kt < 2:
                        # boundary pipelining vs layer-0 evacuation
                        z_t = zp.tile([128, RG], cdt, tag="z", name="zb")
                        for rr in range(2):
                            nc.vector.tensor_mul(
                                z_t[:, rr * NB : (rr + 1) * NB],
                                xcur[0][:, rr * NB : (rr + 1) * NB],
                                h_tiles[(0, half)][:, rr * NB : (rr + 1) * NB],
                            )
                    else:
                        z_t = zp.tile([128, RG], cdt, tag="z", name="zs")
                        nc.vector.tensor_mul(
                            z_t[:, :],
                            xcur[0][:, :],
                            h_tiles[(0, half)][:, g * RG : (g + 1) * RG],
                        )
                    for c in range(2):
                        lhsT = w1[:, kt, c * 128 : (c + 1) * 128]
                        for rr in range(2):
                            nc.tensor.matmul(
                                ps[(c, rr)][:, :],
                                lhsT,
                                z_t[:, rr * NB : (rr + 1) * NB],
                                start=(kt == 0),
                                stop=(kt == KT12 - 1),
                            )
                if pre_evac is not None:
                    pre_evac()
                for rr in range(2):
                    for c in range(2):
                        dst = h_tiles[(1, c)][
                            :, g * RG + rr * NB : g * RG + (rr + 1) * NB
                        ]
                        if c == 0:
                            nc.vector.tensor_scalar_add(
                                dst, ps[(0, rr)][:, :], bias[:, 2:3]
                            )
                        else:
                            nc.scalar.activation(
                                dst,
                                ps[(1, rr)][:, :],
                                mybir.ActivationFunctionType.Identity,
                                bias=bias[:, 3:4],
                            )

            # group 0: weight streaming + small-work hooks.  w2 is split
            # between late group 0 and group 1 (it is consumed only by the
            # very last phase); output DMAs ride the gpsimd software DGE so
            # their data dependencies never stall the two hardware rings.
            w_sched = {0: (w1, w1_p, 1), 3: (w1, w1_p, 2), 8: (w1, w1_p, 3),
                       13: (w1, w1_p, 4), 20: (w1, w1_p, 5)}

            def a_hook(kt):
                if kt == 4:
                    h_reduce(0)   # deferred layer-0 d-sum, off the boundary path
                if kt == 6:
                    nc.gpsimd.dma_start(out_p[:, 0:2, :], out_sb[:, 0:2, :])
                if kt == 30:
                    nc.sync.dma_start(x0d[:, :, :], x0d_p[:, :, :])
                if kt == 50:
                    nc.sync.dma_start(x0d2[:, :, :], x0d2_p[:, :, :])
                if kt == 54:
                    nc.sync.dma_start(ident[:, :], ident_p[:, :])
                if kt == 70:
                    l1_pre[(1, 0)] = make_x(1, 0, "l1xB0")
                if kt == 74:
                    l1_pre[(1, 1)] = make_x(1, 1, "l1xB1")
                if kt in w_sched:
                    wt, wp, c = w_sched[kt]
                    lo = w1_chunks[c]
                    (nc.sync if c % 2 else nc.scalar).dma_start(
                        wt[:, lo : lo + 13, :], wp[:, lo : lo + 13, :]
                    )

            # pre-build group-1's first two z tiles so the PE rolls straight
            # from group 0's last matmul into group 1 (the evacuations and
            # everything downstream then drain in group 1's shadow).
            zB_pre = {}

            def pre_evac_a():
                for kt in range(2):   # both kt 0, 1 use x0 row i=0
                    z_t = zp.tile([128, RG], cdt, tag="z", name=f"zB{kt}")
                    nc.vector.tensor_mul(
                        z_t[:, :],
                        l1_pre[(1, 0)][:, :],
                        h_tiles[(0, kt % 2)][:, RG:],
                    )
                    zB_pre[kt] = z_t

            layer1_group(0, l1_pre, {}, a_hook, pre_evac=pre_evac_a)

            # group 1: group-0's transposes and grams hook into this stream,
            # spaced so the DVE's z-build cadence absorbs each transpose
            st_sched = {2: (0, 0), 8: (0, 1), 14: (0, 2), 20: (0, 3),
                        26: (1, 0), 32: (1, 1), 38: (1, 2), 44: (1, 3)}
            wave_sched = {46: (0, 0), 50: (0, 1), 54: (0, 2), 58: (0, 3),
                          62: (1, 0), 66: (1, 1), 70: (1, 2), 74: (1, 3)}

            w2_sched = {2: 0, 14: 1, 26: 2, 38: 3, 50: 4, 62: 5}

            def b_hook(kt):
                if kt in w2_sched:
                    c = w2_sched[kt]
                    lo = w1_chunks[c]
                    (nc.sync if c % 2 else nc.scalar).dma_start(
                        w2[:, lo : lo + 13, :], w2_p[:, lo : lo + 13, :]
                    )
                if kt in st_sched:
                    h, a = st_sched[kt]
                    emit_st(0, h, a)
                if kt in wave_sched:
                    h, bg = wave_sched[kt]
                    gram_wave(h, bg, filler=0)

            layer1_group(1, l1_pre, zB_pre, b_hook)

            # ---- exposed tail ----
            # h=0 of group 1 via DVE stream transposes; h=1 via PE transposes
            # (the PE is otherwise idle here, and its activity holds the HAM
            # clock at full rate).  The W2 contraction is split h1-first so
            # its first half runs while the DVE still streams h0.
            USE_PE_TRANSPOSE = False
            if USE_PE_TRANSPOSE:
                for a in range(4):
                    emit_st(1, 0, a)
                ptr_tags = ["ps_1_2", "ps_1_3"]
                for k in range(16):
                    ptr = psp.tile(
                        [64, 128], cdt, tag=ptr_tags[k % 2], name=f"ptr{k}"
                    )
                    nc.tensor.transpose(
                        ptr[:, :],
                        h_tiles[(1, 1)][:, RG + 64 * k : RG + 64 * (k + 1)],
                        ident[:, :],
                    )
                    nc.scalar.activation(
                        h2dp[:, k, :], ptr[:, :],
                        mybir.ActivationFunctionType.Identity,
                    )
            else:
                for a in range(4):
                    emit_st(1, 1, a)
                for a in range(4):
                    emit_st(1, 0, a)

            ps_f = [
                psp.tile([128, BL], f32, tag="ps_0_2", name="psf0"),
                psp.tile([128, BL], f32, tag="ps_0_3", name="psf1"),
            ]

            def final_half(hi, h):
                for i in range(F):
                    for uh in range(2):
                        nc.tensor.matmul(
                            ps_f[uh][:, :],
                            w2[:, h * F + i, uh * 128 : (uh + 1) * 128],
                            g2[:, h, i, :],
                            start=(hi == 0 and i == 0),
                            stop=(hi == 1 and i == F - 1),
                        )

            for bg in range(4, 8):
                gram_wave(1, bg, filler=1, pe_path=USE_PE_TRANSPOSE)
            final_half(0, 1)
            for bg in range(4, 8):
                gram_wave(0, bg, filler=2)
            final_half(1, 0)
            h_reduce(1)   # layer-1 d-sum on DVE, overlapping the final matmuls
            nc.gpsimd.dma_start(out_p[:, 2:4, :], out_sb[:, 2:4, :])
            for uh in range(2):
                nc.vector.tensor_copy(out_sb[:, 4 + uh, :], ps_f[uh][:, :])
            nc.sync.dma_start(out_p[:, 4:6, :], out_sb[:, 4:6, :])

    nc.compile()
    return nc


def _get_program():
    if "nc" not in _prog_cache:
        _prog_cache["nc"] = _build_program()
    return _prog_cache["nc"]


def _prep_maps(inputs):
    cdt = _np_dt()
    x = np.asarray(inputs["inputs"], np.float32)          # [512, 39, 32]
    Ws = [np.asarray(inputs[f"W{k}"], np.float32) for k in range(3)]
    bs = [np.asarray(inputs[f"b{k}"], np.float32) for k in range(3)]

    ii, jj = np.triu_indices(F)                           # 780 pairs, i-major

    # layer-0 weights: symmetric fold, pair p -> tile p//128, partition p%128
    w0r = Ws[0].reshape(F, F, U)
    w0s = np.where((ii == jj)[:, None], w0r[ii, jj], w0r[ii, jj] + w0r[jj, ii])
    w0t = np.zeros((KT0 * 128, U), np.float32)
    w0t[:NP] = w0s
    w_tiled = [
        w0t.reshape(KT0, 128, U).transpose(1, 0, 2).astype(cdt),
        Ws[1].reshape(KT12, 128, U).transpose(1, 0, 2).astype(cdt),
        # W2 relayout for the gram contraction: [(i, j), u] ->
        # [j%128, (j//128)*F + i, u]
        Ws[2].reshape(F, 2, 128, U).transpose(2, 1, 0, 3).reshape(128, KT12, U)
        .astype(cdt),
    ]
    w_tiled = [np.ascontiguousarray(w) for w in w_tiled]
    bias = np.zeros((128, 4), np.float32)
    for l in range(2):
        for c in range(2):
            bias[:, l * 2 + c] = bs[l][c * 128 : (c + 1) * 128]

    in_maps = []
    for core in range(N_CORES):
        xs = x[core * BL : (core + 1) * BL]               # [64, 39, 32]
        x0T = xs.transpose(1, 0, 2).reshape(F, R)         # fp32 [39, 2048]
        z0 = np.zeros((KT0 * 128, R), np.float32)
        z0[:NP] = x0T[ii] * x0T[jj]
        z0t = np.ascontiguousarray(
            z0.reshape(KT0, 128, R).transpose(1, 0, 2).astype(cdt)
        )
        x0r = np.ascontiguousarray(np.repeat(x0T.astype(cdt), 32, axis=0))
        x0d = np.ascontiguousarray(xs.transpose(2, 0, 1).astype(cdt))
        # x0d2[(b%2)*32 + d, b//2, i] = xs[b, i, d]
        x0d2 = np.ascontiguousarray(
            xs.reshape(32, 2, F, D).transpose(1, 3, 0, 2).reshape(64, 32, F)
            .astype(cdt)
        )
        in_maps.append(
            {
                "z0": z0t,
                "x0r": x0r,
                "x0d": x0d,
                "x0d2": x0d2,
                "ident": np.eye(128, dtype=cdt),
                "w0": w_tiled[0],
                "w1": w_tiled[1],
                "w2": w_tiled[2],
                "bias": bias,
            }
        )
    return in_maps, bs


def _finish_output(results, bs):
    outs = []
    for core in range(N_CORES):
        o = np.asarray(results[core]["out"], np.float32)  # [128, 6, 64]
        outs.append(o.transpose(2, 1, 0).reshape(BL, 768))
    out = np.concatenate(outs, axis=0)
    for l in range(3):
        out[:, l * U : (l + 1) * U] += D * bs[l]
    return np.ascontiguousarray(out.astype(np.float32))


def kernel(**inputs) -> np.ndarray:
    from concourse.bass_utils import run_bass_kernel_spmd

    in_maps, bs = _prep_maps(inputs)
    nc = _get_program()
    res = run_bass_kernel_spmd(nc, in_maps, list(range(N_CORES))).results
    return _finish_output(res, bs)
